# revision 8
# baseline (speedup 1.0000x reference)
"""Trainium2 Bass kernel for nn_AttentionModel (histogram_binning).

Pipeline per core (rows sharded across 8 cores, 32768 rows each):
  x = [x_lf, x_l] (N, 256)
  h = tanh(x @ W1 + b1); z = h @ W2 + b2          -- PE matmuls in fp32r,
     computed transposed (features on partitions) so weights load naturally;
     X^T obtained with PE transposes.
  e = exp(z)  (softmax numerator; z is bounded so no max subtraction needed)
  sigma = row-sum(e) via ACT accumulate during the PSUM->SBUF copy
  coverage = e * (1/sigma) * (x_lf >= 0)          -- fused DVE scalar_tensor_tensor
  hist_c = sum_r e_r * (x_lf_r == c), c = 0..8    -- fused masked reduces
  q_c = hist_c / sigma; q_9 from the coverage row-sum (sum_c q_c identity)
  out1 = q - ln(sum_c exp(q_c))                   -- deferred log_softmax phase
"""

import sys

sys.path.insert(0, "/opt/trn_rl_repo")

from contextlib import ExitStack

import numpy as np

import concourse.bass as bass
import concourse.mybir as mybir
import concourse.tile as tile
from concourse import masks
from concourse.bass_utils import run_bass_kernel_spmd
from concourse.vector_clock import ScopedClock

FP = mybir.dt.float32
FR = mybir.dt.float32r
AF = mybir.ActivationFunctionType
OP = mybir.AluOpType

N = 262144
N_CORES = 8
NPC = N // N_CORES  # rows per core
ST = 512  # supertile rows
NCLASS = 10

# Number of histogram classes computed with explicit masked reduces; the last
# class is recovered from the coverage row-sum identity sum_c q_c = cov_rowsum.
NEXPL = NCLASS - 1
# How many of the explicit classes run on GPSIMD (rest on DVE).
N_ON_GPSIMD = 0


def _patched_drain_and_barrier(self, tick_clock, wait_clock):
    # The stock tail drain attaches every outstanding semaphore wait to one
    # instruction; this walrus build rejects >1 sync wait per SP instruction.
    # Split the waits across a chain of drains.
    nc = self.nc
    drain_inst = nc.sync.drain()
    wait_clock.add_sem_waits(
        drain_inst.ins, ScopedClock({None: tick_clock.global_clock})
    )
    si = drain_inst.ins.sync_info
    waits = list(si.on_wait or [])
    if len(waits) > 1:
        si.on_wait = [waits[0]]
        for w in waits[1:]:
            extra = nc.sync.drain()
            esi = extra.ins.sync_info
            if esi is None:
                extra.ins.sync_info = mybir.SyncInfo(on_wait=[w], on_update=[])
            else:
                esi.on_wait = [w]
    nc.all_engine_barrier()
    popped = nc._tile_sem_poison_stack.pop()
    assert popped is self._sem_poison
    nc.clear_and_free_semaphores(list(self.sems.allocated().values()))
    nc.all_engine_barrier()


tile.TileContext._drain_and_barrier = _patched_drain_and_barrier


def _split_sync_waits(nc, max_waits=1):
    """This walrus build rejects instructions carrying more than one sync
    wait; hoist extras onto NoOps inserted just before the instruction."""
    ctr = 0
    for fn in nc.m.functions:
        for bb in fn.blocks:
            out = []
            for inst in bb.instructions:
                si = inst.sync_info
                if si is not None and si.on_wait and len(si.on_wait) > max_waits:
                    waits = list(si.on_wait)
                    si.on_wait = waits[: max_waits]
                    rest = waits[max_waits:]
                    for i in range(0, len(rest), max_waits):
                        nop = mybir.InstNoOp(name=f"_waitnop_{ctr}", ins=[], outs=[])
                        ctr += 1
                        nop.engine = inst.engine
                        nop.sync_info = mybir.SyncInfo(
                            on_wait=rest[i : i + max_waits], on_update=[]
                        )
                        out.append(nop)
                out.append(inst)
            bb.instructions[:] = out


def build_program(npc=NPC, split_waits=True):
    assert npc % ST == 0
    nst = npc // ST  # supertiles per core
    ntiles = npc // 128  # 128-row tiles per core
    tpst = ST // 128  # tiles per supertile

    nc = bass.Bass()
    x_lf = nc.declare_dram_parameter("x_lf", [npc, 128], FP, isOutput=False)
    x_l = nc.declare_dram_parameter("x_l", [npc, 128], FP, isOutput=False)
    w1 = nc.declare_dram_parameter("W1", [256, 256], FP, isOutput=False)
    b1 = nc.declare_dram_parameter("b1", [256, 1], FP, isOutput=False)
    w2 = nc.declare_dram_parameter("W2", [256, 128], FP, isOutput=False)
    b2 = nc.declare_dram_parameter("b2", [128, 1], FP, isOutput=False)
    cov = nc.declare_dram_parameter("cov", [npc, 128], FP, isOutput=True)
    out1 = nc.declare_dram_parameter("out1", [npc, NCLASS], FP, isOutput=True)

    with tile.TileContext(nc) as tc, ExitStack() as ctx:
        consts = ctx.enter_context(tc.tile_pool(name="consts", bufs=1))
        stage = ctx.enter_context(tc.tile_pool(name="stage", bufs=1))
        inp = ctx.enter_context(tc.tile_pool(name="inp", bufs=3))
        mid = ctx.enter_context(tc.tile_pool(name="mid", bufs=2))
        outp = ctx.enter_context(tc.tile_pool(name="outp", bufs=3))
        small = ctx.enter_context(tc.tile_pool(name="small", bufs=2))
        scr = ctx.enter_context(tc.tile_pool(name="scr", bufs=3))
        psum2 = ctx.enter_context(tc.tile_pool(name="psum2", bufs=1, space="PSUM"))

        ident = consts.tile([128, 128], FP)
        masks.make_identity(nc, ident[:])

        w1sb = []
        b1sb = []
        for k in range(2):
            t0 = consts.tile([128, 256], FP, tag=f"w1raw_{k}", name=f"w1raw_{k}")
            nc.sync.dma_start(t0[:], w1[k * 128 : (k + 1) * 128, :])
            t = consts.tile([128, 256], FR, tag=f"w1_{k}", name=f"w1_{k}")
            nc.scalar.copy(t[:], t0[:])
            w1sb.append(t)
            tb = consts.tile([128, 1], FP, tag=f"b1_{k}", name=f"b1_{k}")
            nc.sync.dma_start(tb[:], b1[k * 128 : (k + 1) * 128, :])
            b1sb.append(tb)
        w2sb = []
        for k in range(2):
            t0 = consts.tile([128, 128], FP, tag=f"w2raw_{k}", name=f"w2raw_{k}")
            nc.sync.dma_start(t0[:], w2[k * 128 : (k + 1) * 128, :])
            t = consts.tile([128, 128], FR, tag=f"w2_{k}", name=f"w2_{k}")
            nc.scalar.copy(t[:], t0[:])
            w2sb.append(t)
        b2sb = consts.tile([128, 1], FP)
        nc.sync.dma_start(b2sb[:], b2[:, :])

        q_st = stage.tile([128, ntiles, NCLASS], FP)
        es_st = stage.tile([128, ntiles], FP)

        for s in range(nst):
            r0 = s * ST

            xlf_t = inp.tile([128, tpst, 128], FP, tag="xlf")
            nc.sync.dma_start(
                xlf_t[:],
                x_lf[r0 : r0 + ST, :].rearrange("(t p) f -> p t f", p=128),
            )
            xl_t = inp.tile([128, tpst, 128], FP, tag="xl")
            nc.sync.dma_start(
                xl_t[:],
                x_l[r0 : r0 + ST, :].rearrange("(t p) f -> p t f", p=128),
            )

            # X^T via PE transposes (fp32, exact), then PSUM -> SBUF on ACT.
            xt_ps = [
                psum2.tile([128, ST], FP, tag="xt0", name="xt0_ps", bufs=1),
                psum2.tile([128, ST], FP, tag="xt1", name="xt1_ps", bufs=1),
            ]
            for t in range(tpst):
                nc.tensor.transpose(
                    xt_ps[0][:, t * 128 : (t + 1) * 128], xlf_t[:, t], ident[:]
                )
                nc.tensor.transpose(
                    xt_ps[1][:, t * 128 : (t + 1) * 128], xl_t[:, t], ident[:]
                )
            xt_sb = [
                mid.tile([128, ST], FR, tag="xt0sb", name="xt0_sb"),
                mid.tile([128, ST], FR, tag="xt1sb", name="xt1_sb"),
            ]
            nc.scalar.copy(xt_sb[0][:], xt_ps[0][:])
            nc.scalar.copy(xt_sb[1][:], xt_ps[1][:])

            # H^T = tanh(W1^T X^T + b1): hid chunk h on partitions.
            ht = []
            for h in range(2):
                h_ps = psum2.tile([128, ST], FP, tag=f"h{h}", bufs=1)
                for k in range(2):
                    nc.tensor.matmul(
                        h_ps[:],
                        w1sb[k][:, h * 128 : (h + 1) * 128],
                        xt_sb[k][:],
                        start=(k == 0),
                        stop=(k == 1),
                    )
                ht_sb = mid.tile([128, ST], FR, tag=f"ht{h}")
                nc.scalar.activation(ht_sb[:], h_ps[:], AF.Tanh, bias=b1sb[h][:])
                ht.append(ht_sb)

            # Z^T = W2^T H^T + b2 (bias folded into the exp)
            z_ps = psum2.tile([128, ST], FP, tag="z", bufs=2)
            for k in range(2):
                nc.tensor.matmul(
                    z_ps[:],
                    w2sb[k][:],
                    ht[k][:],
                    start=(k == 0),
                    stop=(k == 1),
                )
            eT = mid.tile([128, ST], FP, tag="eT")
            nc.scalar.activation(eT[:], z_ps[:], AF.Exp, bias=b2sb[:])

            # transpose exp(z) back to rows-on-partitions
            e_ps = psum2.tile([128, ST], FP, tag="e", bufs=2)
            for t in range(tpst):
                nc.tensor.transpose(
                    e_ps[:, t * 128 : (t + 1) * 128],
                    eT[:, t * 128 : (t + 1) * 128],
                    ident[:],
                )
            e_sb = mid.tile([128, tpst, 128], FP, tag="e_sb")
            sig = small.tile([128, tpst], FP, tag="sig")
            for t in range(tpst):
                nc.scalar.activation(
                    e_sb[:, t],
                    e_ps[:, t * 128 : (t + 1) * 128],
                    AF.Copy,
                    accum_out=sig[:, t : t + 1],
                )

            rs = small.tile([128, tpst], FP, tag="rs")
            nc.vector.reciprocal(rs[:], sig[:])

            mask_t = outp.tile([128, tpst, 128], FP, tag="mask")
            for t in range(tpst):
                nc.gpsimd.tensor_scalar(
                    mask_t[:, t], xlf_t[:, t], 0.0, None, OP.is_ge
                )

            cov_t = outp.tile([128, tpst, 128], FP, tag="cov")
            covsum = small.tile([128, tpst], FP, tag="covsum")
            for t in range(tpst):
                nc.vector.scalar_tensor_tensor(
                    cov_t[:, t],
                    e_sb[:, t],
                    rs[:, t : t + 1],
                    mask_t[:, t],
                    OP.mult,
                    OP.mult,
                    accum_out=covsum[:, t : t + 1],
                )
            nc.sync.dma_start(
                cov[r0 : r0 + ST, :].rearrange("(t p) f -> p t f", p=128),
                cov_t[:],
            )

            # histogram: hist[:, t, c] = sum_r e * (x_lf == c)
            hist = small.tile([128, tpst, NEXPL], FP, tag="hist")
            for t in range(tpst):
                for c in range(NEXPL):
                    if c < NEXPL - N_ON_GPSIMD:
                        eng = nc.vector
                        s_t = scr.tile([128, 128], FP, tag="scr_v", name="scr_v")
                    else:
                        eng = nc.gpsimd
                        s_t = scr.tile([128, 128], FP, tag="scr_g", name="scr_g")
                    eng.scalar_tensor_tensor(
                        s_t[:],
                        xlf_t[:, t],
                        float(c),
                        e_sb[:, t],
                        OP.is_equal,
                        OP.mult,
                        accum_out=hist[:, t, c : c + 1],
                    )

            # q_c = hist_c / sigma for c<9; q_9 = covsum - sum_{c<9} q_c
            qs = q_st[:, s * tpst : (s + 1) * tpst, :]
            for t in range(tpst):
                nc.vector.tensor_scalar(
                    qs[:, t, 0:NEXPL], hist[:, t], rs[:, t : t + 1], None, OP.mult
                )
            qsum = small.tile([128, tpst], FP, tag="qsum")
            nc.vector.tensor_reduce(
                qsum[:], qs[:, :, 0:NEXPL], axis=mybir.AxisListType.X, op=OP.add
            )
            nc.vector.tensor_tensor(
                qs[:, :, NEXPL : NEXPL + 1],
                covsum[:].broadcast_to([128, tpst, 1]),
                qsum[:].broadcast_to([128, tpst, 1]),
                OP.subtract,
            )

            # stash sum_c exp(q_c) for the deferred log-softmax
            eq = scr.tile([128, tpst, NCLASS], FP, tag="eq")
            nc.scalar.activation(eq[:], qs[:], AF.Exp)
            nc.vector.tensor_reduce(
                es_st[:, s * tpst : (s + 1) * tpst],
                eq[:],
                axis=mybir.AxisListType.X,
                op=OP.add,
            )

        # phase 2: out1 = q - ln(sum_c exp(q_c))
        lse = stage.tile([128, ntiles], FP)
        nc.scalar.activation(lse[:], es_st[:], AF.Ln)
        o_sb = stage.tile([128, ntiles, NCLASS], FP)
        nc.vector.tensor_tensor(
            o_sb[:],
            q_st[:],
            lse[:].broadcast_to([128, ntiles, NCLASS]),
            OP.subtract,
        )
        nc.sync.dma_start(
            out1[:, :].rearrange("(t p) c -> p t c", p=128), o_sb[:]
        )

    if split_waits:
        _split_sync_waits(nc)
    return nc


_prog_cache = {}


def _get_program(npc):
    if npc not in _prog_cache:
        _prog_cache[npc] = build_program(npc)
    return _prog_cache[npc]


def kernel(x_lf, x_l, W1, b1, W2, b2):
    n = x_lf.shape[0]
    npc = n // N_CORES
    nc = _get_program(npc)
    in_maps = []
    for i in range(N_CORES):
        sl = slice(i * npc, (i + 1) * npc)
        in_maps.append(
            {
                "x_lf": np.ascontiguousarray(x_lf[sl]),
                "x_l": np.ascontiguousarray(x_l[sl]),
                "W1": np.ascontiguousarray(W1),
                "b1": np.ascontiguousarray(b1.reshape(256, 1)),
                "W2": np.ascontiguousarray(W2),
                "b2": np.ascontiguousarray(b2.reshape(128, 1)),
            }
        )
    res = run_bass_kernel_spmd(nc, in_maps, core_ids=list(range(N_CORES)))
    out1 = np.concatenate([res.results[i]["out1"] for i in range(N_CORES)], axis=0)
    covr = np.concatenate([res.results[i]["cov"] for i in range(N_CORES)], axis=0)
    return out1, covr
